# revision 74
# baseline (speedup 1.0000x reference)
"""Multi-head causal self-attention (B=2, S=2048, E=1024, H=16, D=64) on 8
Trainium2 NeuronCores.

Sharding: batch x head-group. Core c handles batch (c // 4) and heads
[4*(c%4), 4*(c%4)+4). Each core computes QKV projection for its 4 heads,
causal flash-attention, and a partial output projection over its head
columns. Host sums the 4 partial outputs per batch and adds b_out.

QKV projections run as split-fp8 DoubleRow matmuls: x and the (pre-scaled)
weights are sent as fp8 (hi, lo) pairs with t ~ t_hi + t_lo, the lo part
holding the fp8 rounding residual at natural scale. One PSUM group of 12
DoubleRow (K=256) matmuls accumulates xh*wh + xh*wl + xl*wh (the dropped
xl*wl term is ~1e-6), recovering the product to ~1e-3 relative error
(better than bf16) at 0.75x the bf16 matmul cost; staging out of PSUM is a
plain copy. Attention (scores, exp, ctx, out-proj) stays bf16: an ablation
sweep shows plain-fp8 noise at ANY single attention stage alone costs
~2.5e-2 output error, over the 2e-2 tolerance. Output partials are stored
bf16 (halves the output DMA); the host sums them in float64.

Hardware constraints the cost model does not enforce: GPSIMD (Pool) cannot
touch PSUM; no DVE op may read two PSUM operands; weight DMAs need >=512B
contiguous runs per partition to avoid a 2x latency penalty (hence the
p-major [128, 8*256] weight layout); and PSUM banks must not be read while
later matmuls still accumulate into them (a column-interleaved tail that
did this passed TimelineSim + CoreSim but corrupted results on silicon).

Attention uses the transpose-free S^T formulation: scores S^T[k, q] =
kT.T @ qT (K = head dim = 64), softmax denominators ride as an appended
ones-column on V (M = 65 ctx matmul), normalization divides by the
denominator row broadcast across partitions on gpsimd. Heads are
processed in pairs with scores/exp/ctx software-pipelined, and the next
chunk's QKV projection + previous chunk's out-projection are interleaved
into the attention waves to keep the PE busy while ACT runs exp.
"""

import sys

if "/opt/trn_rl_repo" not in sys.path:
    sys.path.insert(0, "/opt/trn_rl_repo")

import numpy as np
import ml_dtypes

import concourse.bacc as bacc
import concourse.mybir as mybir
import concourse.tile as tile

BF16 = mybir.dt.bfloat16
FP32 = mybir.dt.float32
FP8 = mybir.dt.float8e4
DR = mybir.MatmulPerfMode.DoubleRow

B, S, E = 2, 2048, 1024
H, DH = 16, 64
NCORES = 8
HPC = 4            # heads per core
M = HPC * DH       # 256 qkv columns per core
QC = 512           # q chunk
KB = 128           # k block
NSC = S // QC      # 4 s-chunks
WS = 32.0          # fp8 weight pre-scale (powers of 2, exact)
LS = 64.0          # fp8 lo-part scale
SCALE = (1.0 / np.sqrt(DH)) / (WS * WS)  # exp scale on 1024x-scaled scores


def _emit_kernel(tc, xh_d, xl_d, whq_d, whk_d, whv_d, wlq_d, wlk_d, wlv_d, wo_d, out):
    nc = tc.nc
    Exp = mybir.ActivationFunctionType.Exp
    Mult = mybir.AluOpType.mult
    Add = mybir.AluOpType.add

    with tc.tile_pool(name="res", bufs=1) as res, \
         tc.tile_pool(name="ps", bufs=1, space="PSUM") as ps, \
         tc.tile_pool(name="expp", bufs=6) as expp, \
         tc.tile_pool(name="scr", bufs=2) as scr, \
         tc.tile_pool(name="outb", bufs=2) as outb:

        # ---- resident SBUF tiles ----
        # x hi/lo: [p, e, s] fp8 (single-DMA chunked loads)
        xth = res.tile([128, 8, S], FP8, name="xth")
        xtl = res.tile([128, 8, S], FP8, name="xtl")
        # weights hi/lo: [p, e, 256] fp8 per projection (separate tiles so
        # each DMA is a contiguous 2KB-per-partition transfer, avoiding the
        # sub-512B-run DMA latency penalty of column-sliced loads)
        whq = res.tile([128, 8, 256], FP8, name="whq")
        whk = res.tile([128, 8, 256], FP8, name="whk")
        whv = res.tile([128, 8, 256], FP8, name="whv")
        wlq = res.tile([128, 8, 256], FP8, name="wlq")
        wlk = res.tile([128, 8, 256], FP8, name="wlk")
        wlv = res.tile([128, 8, 256], FP8, name="wlv")
        wot = [res.tile([128, E], BF16, name=f"wot{i}") for i in range(2)]
        qTt = [res.tile([128, S], BF16, name=f"qTt{i}") for i in range(2)]
        kTt = [res.tile([128, S], BF16, name=f"kTt{i}") for i in range(2)]
        ctxT = [res.tile([128, S], BF16, name=f"ctxT{i}") for i in range(2)]
        # V with ones column: per (k-block kb, head h) a [128, 65] slab
        v1 = res.tile([128, (S // KB) * HPC * 65], BF16, name="v1")
        v1_3d = v1.rearrange("p (n c) -> p n c", c=65)
        mask = res.tile([128, 128], BF16, name="mask")

        # ---- input DMA: one batched transfer per tensor/chunk. The split
        # proj A-chain needs only (wh, xh): those go first so the first
        # matmuls start as early as possible.
        xh_3d = xh_d.rearrange("(e p) s -> p e s", p=128)
        xl_3d = xl_d.rearrange("(e p) s -> p e s", p=128)
        wsrc = {t.name.split("_")[0]: t.rearrange("p (e c) -> p e c", c=256)
                for t in (whq_d, whk_d, whv_d, wlq_d, wlk_d, wlv_d)}
        # startup latency order: the chunk-0 q unit (A then B chains) is the
        # first consumer, then k, then v; lo parts right after their hi.
        nc.sync.dma_start(whq[:], wsrc["whq"])
        nc.sync.dma_start(xth[:, 0:4, 0:QC], xh_3d[:, 0:4, 0:QC])
        nc.sync.dma_start(xth[:, 4:8, 0:QC], xh_3d[:, 4:8, 0:QC])
        nc.sync.dma_start(wlq[:], wsrc["wlq"])
        nc.sync.dma_start(xtl[:, 0:4, 0:QC], xl_3d[:, 0:4, 0:QC])
        nc.sync.dma_start(xtl[:, 4:8, 0:QC], xl_3d[:, 4:8, 0:QC])
        nc.sync.dma_start(whk[:], wsrc["whk"])
        nc.sync.dma_start(wlk[:], wsrc["wlk"])
        nc.sync.dma_start(whv[:], wsrc["whv"])
        nc.sync.dma_start(wlv[:], wsrc["wlv"])
        for chunk in range(1, NSC):
            cs = slice(chunk * QC, (chunk + 1) * QC)
            nc.sync.dma_start(xth[:, :, cs], xh_3d[:, :, cs])
            nc.sync.dma_start(xtl[:, :, cs], xl_3d[:, :, cs])
        for i in range(2):
            nc.sync.dma_start(wot[i][:], wo_d[i * 128:(i + 1) * 128, :])

        # ---- constants ----
        nc.gpsimd.memset(v1[:], 1.0)  # data columns overwritten by V proj
        # stair mask: keep where k_local <= q_local (within a 128x128 block)
        nc.gpsimd.memset(mask[:], 1.0)
        nc.gpsimd.affine_select(
            out=mask[:], in_=mask[:],
            compare_op=mybir.AluOpType.is_ge,
            fill=0.0, base=0,
            pattern=[[1, 128]],
            channel_multiplier=-1,
        )

        # ---- emission helpers ----
        def emit_proj_qk(sc, mt, whx, wlx, dstt, kind):
            # x*w with x ~ xh + xl, w ~ wh + wl (lo parts at natural scale):
            # one PSUM group of 12 DR matmuls (K=256 each) accumulates
            # xh*wh + xh*wl + xl*wh; the dropped xl*wl term is ~1e-6.
            s0 = sc * QC
            cs = slice(mt * 128, (mt + 1) * 128)
            pq = ps.tile([128, QC], FP32, tag="proj", bufs=2,
                         name=f"pq{kind}_{sc}_{mt}")
            # hi*hi terms first: at kernel start they only need the first
            # (wh, xh) DMAs, so the PE starts ~2 transfers earlier
            for i, (wt, xt) in enumerate(
                    ((whx, xth), (wlx, xth), (whx, xtl))):
                for j in range(4):
                    es = slice(2 * j, 2 * j + 2)
                    nc.tensor.matmul(
                        pq[:], lhsT=wt[:, es, cs], rhs=xt[:, es, s0:s0 + QC],
                        start=(i == 0 and j == 0), stop=(i == 2 and j == 3),
                        perf_mode=DR)
            nc.vector.tensor_copy(dstt[mt][:, s0:s0 + QC], pq[:])

        def emit_proj_v(sc, sb):
            sblk = sc * 4 + sb
            ss = slice(sblk * 128, (sblk + 1) * 128)
            pv = ps.tile([128, M], FP32, tag="proj", bufs=2,
                         name=f"pv_{sblk}")
            for i, (xt, wt) in enumerate(
                    ((xth, whv), (xtl, whv), (xth, wlv))):
                for j in range(4):
                    es = slice(2 * j, 2 * j + 2)
                    nc.tensor.matmul(
                        pv[:], lhsT=xt[:, es, ss], rhs=wt[:, es, :],
                        start=(i == 0 and j == 0), stop=(i == 2 and j == 3),
                        perf_mode=DR)
            nc.vector.tensor_copy(
                v1_3d[:, sblk * HPC:(sblk + 1) * HPC, 0:64],
                pv[:].rearrange("p (h c) -> p h c", c=64))

        def proj_qk_pieces(sc):
            pcs = []
            for mt in range(2):
                pcs.append(lambda mt=mt: emit_proj_qk(sc, mt, whq, wlq,
                                                      qTt, "q"))
                pcs.append(lambda mt=mt: emit_proj_qk(sc, mt, whk, wlk,
                                                      kTt, "k"))
            return pcs

        def proj_v_pieces(sc):
            return [lambda sb=sb: emit_proj_v(sc, sb) for sb in range(4)]

        ob_tiles = {}
        out_4d = out.rearrange("(c q p) f -> p c q f", p=128, q=4)

        def emit_outproj(qb, fc):
            sc, qq = qb // 4, qb % 4
            last = sc == NSC - 1
            if (qq == 0 or last) and fc == 0:
                # chunks 0-2: one staging tile + one batched store per chunk
                # (HWDGE generation cost). Chunk 3 stores per q-block so the
                # final store isn't one big exposed transfer at kernel end.
                shape = [128, E] if last else [128, 4 * E]
                ob_tiles[sc] = outb.tile(shape, BF16, tag="ob",
                                         name=f"ob_{qb}")
            ob = ob_tiles[sc]
            po = ps.tile([128, QC], FP32, tag="proj", bufs=2, name=f"po_{qb}_{fc}")
            for mc in range(2):
                nc.tensor.matmul(
                    po[:],
                    lhsT=ctxT[mc][:, qb * 128:(qb + 1) * 128],
                    rhs=wot[mc][:, fc * QC:(fc + 1) * QC],
                    start=(mc == 0), stop=(mc == 1))
            qoff = 0 if last else qq * E
            if last:
                # ACT is idle once the last exp has drained; doing the final
                # staging copies there keeps the tail's DVE free for the
                # norm chains (Pool can't touch PSUM on real hardware)
                nc.scalar.copy(
                    ob[:, qoff + fc * QC: qoff + (fc + 1) * QC], po[:])
            else:
                nc.vector.tensor_copy(
                    ob[:, qoff + fc * QC: qoff + (fc + 1) * QC], po[:])
            if last:
                nc.sync.dma_start(
                    out[qb * 128:(qb + 1) * 128, fc * QC:(fc + 1) * QC],
                    ob[:, fc * QC:(fc + 1) * QC])
                if fc == 1:
                    del ob_tiles[sc]
            elif (qq, fc) == (3, 1):
                ob4 = ob.rearrange("p (q f) -> p q f", f=E)
                nc.sync.dma_start(out_4d[:, sc, 0:2, :], ob4[:, 0:2, :])
                nc.sync.dma_start(out_4d[:, sc, 2:4, :], ob4[:, 2:4, :])
                del ob_tiles[sc]

        def outproj_pieces(sc):
            return [lambda qb=qb, fc=fc: emit_outproj(qb, fc)
                    for qb in range(sc * 4, sc * 4 + 4) for fc in range(2)]

        # ---- attention waves (one head PAIR, one k-block) ----
        # The pair's two heads live on complementary partition halves of the
        # same qT/kT tile (rows 0-63 and 64-127), so their K=64 scores
        # matmuls land on disjoint PE row-groups (tile_position (0,0) and
        # (64,0)) and execute concurrently on hardware.
        def wave_scores(sc, pair, kb):
            s0 = sc * QC
            mt = pair
            rel = kb - 4 * sc
            # Diagonal-adjacent blocks (rel >= 1) compute/exp only the
            # columns at and above the diagonal, kept at their natural
            # offsets - the two heads' outputs must stay in SEPARATE PSUM
            # banks (the row-tiled score matmuls execute concurrently;
            # same-bank concurrent PE writes fault on hardware).
            lo = rel * 128 if rel >= 1 else 0
            sc_ps = ps.tile([128, 2 * QC], FP32, tag="scores", bufs=2,
                            name=f"s_{sc}_{pair}_{kb}")
            for hh in range(2):
                r0 = hh * 64
                off = hh * QC
                nc.tensor.matmul(
                    sc_ps[:, off + lo: off + QC],
                    lhsT=kTt[mt][r0:r0 + 64, kb * 128:(kb + 1) * 128],
                    rhs=qTt[mt][r0:r0 + 64, s0 + lo: s0 + QC],
                    start=True, stop=True)
            ex = expp.tile([128, 2 * QC], BF16, tag="ex",
                           name=f"e_{sc}_{pair}_{kb}")
            if lo:
                # one strided 3D-AP call over both heads' live columns
                ex3 = ex.rearrange("p (h q) -> p h q", q=QC)
                sp3 = sc_ps.rearrange("p (h q) -> p h q", q=QC)
                nc.scalar.activation(ex3[:, :, lo:QC], sp3[:, :, lo:QC],
                                     Exp, scale=SCALE)
            else:
                nc.scalar.activation(ex[:], sc_ps[:], Exp, scale=SCALE)
            if rel >= 0:
                for hh in range(2):
                    off = hh * QC
                    nc.vector.tensor_mul(
                        ex[:, off + rel * 128: off + (rel + 1) * 128],
                        ex[:, off + rel * 128: off + (rel + 1) * 128],
                        mask[:])
            return ex

        def wave_ctx(sc, pair, kb, ex, ctx_pair, nkb):
            rel = kb - 4 * sc
            lo = rel * 128 if rel > 0 else 0
            for hh in range(2):
                h = 2 * pair + hh
                off = hh * QC
                nc.tensor.matmul(
                    ctx_pair[hh][:, lo:QC],
                    lhsT=v1_3d[:, kb * HPC + h, :],
                    rhs=ex[:, off + lo: off + QC],
                    start=(kb == 0), stop=(kb == nkb - 1),
                    skip_group_check=True)

        def emit_norm(sc, h, ctx_ps, stage=True, cols=None, rec=None,
                      mul_eng=None):
            s0 = sc * QC
            mt, r0 = h // 2, (h % 2) * 64
            c0, cw = (0, QC) if cols is None else cols
            # recip reads the PSUM denom row directly so it doesn't chain
            # behind the staging copy
            if rec is None:
                rec = scr.tile([1, QC], FP32, tag="rec", name=f"r_{sc}_{h}_{c0}")
                nc.vector.reciprocal(rec[:, c0:c0 + cw],
                                     ctx_ps[64:65, c0:c0 + cw])
            if stage:
                # stage the accumulated ctx out of PSUM immediately: the
                # PSUM slot is recycled by the next head pair, and holding
                # it through the recip -> gpsimd-broadcast chain stalls the
                # next chunk's first ctx matmuls at every chunk boundary.
                cst = scr.tile([65, QC], FP32, tag="cst", name=f"cs_{sc}_{h}")
                nc.vector.tensor_copy(cst[:], ctx_ps[:])
                src = cst
            else:
                # final pair of the kernel: nobody reuses the slot, skip the
                # copy to shorten the tail chain
                src = ctx_ps
            recb = scr.tile([64, QC], FP32, tag="recb", name=f"rb_{sc}_{h}_{c0}")
            nc.gpsimd.partition_broadcast(recb[:, c0:c0 + cw],
                                          rec[:, c0:c0 + cw])
            (mul_eng or nc.vector).tensor_mul(
                ctxT[mt][r0:r0 + 64, s0 + c0:s0 + c0 + cw],
                src[0:64, c0:c0 + cw], recb[:, c0:c0 + cw])

        # ---- main schedule ----
        # exp-table warm: first ACT exp pays the ~2.7us table load; issue a
        # tiny one immediately so it overlaps the initial DMA.
        warm = scr.tile([1, 1], FP32, tag="warm", bufs=1, name="warm")
        nc.gpsimd.memset(warm[:], 0.0)
        nc.scalar.activation(warm[:], warm[:], Exp)

        # only chunk 0's mt=0 q/k projections run serially before the first
        # attention wave (DMA-gated anyway); mt=1 and V are pinned inside
        # chunk 0's waves below.
        emit_proj_qk(0, 0, whq, wlq, qTt, "q")
        emit_proj_qk(0, 0, whk, wlk, kTt, "k")
        for pc in proj_v_pieces(0):
            pc()
        pending_norms = []
        for sc in range(NSC):
            nkb = 4 * (sc + 1)
            waves = [(pair, kb) for pair in range(2)
                     for kb in range(nkb)]
            # Filler distribution. Only chunk c's mt=0 q/k projections must
            # strictly precede attention(c) (its pair-0 scores use them at
            # wave 0); mt=1 q/k feed pair 1 (wave nkb) and V feeds ctx from
            # wave 4c, so both are head-pinned INSIDE attention(c) itself.
            # This shifts filler from the early (PE-bound) chunks into the
            # later ACT-paced stretches.
            head = []   # pieces pinned to the earliest waves, one per wave
            extra = []  # pieces distributed evenly over all waves
            pins = {}   # wave -> pieces with exact placement constraints
            if sc == 0:
                # mt=1 q/k is only needed by pair 1's scores at wave 4
                qk0 = proj_qk_pieces(0)
                pins = {0: [qk0[2]], 1: [qk0[3]]}
                extra += proj_qk_pieces(1)[:2] + proj_v_pieces(1)
            elif sc == 1:
                head += proj_qk_pieces(1)[2:]
                extra += proj_qk_pieces(2)[:2] + proj_v_pieces(2)
            elif sc == 2:
                head += proj_qk_pieces(2)[2:]
                extra += proj_qk_pieces(3)[:2] + proj_v_pieces(3) + \
                    outproj_pieces(0)[:4]
            else:
                head += proj_qk_pieces(3)[2:]
                extra += outproj_pieces(0)[4:] + outproj_pieces(1) + \
                    outproj_pieces(2)
            # 'extra' pieces include out-projections that read ctxT, which
            # (coarse tile deps) wait on the previous chunk's normalization
            # chain - starting them a few waves in keeps them off the PE's
            # in-order critical path at the chunk boundary.
            sched = {w: [] for w in range(len(waves))}
            for w, pcs in pins.items():
                sched[w].extend(pcs)
            for j, pc in enumerate(head):
                sched[j].append(pc)
            if extra:
                w0 = min(1, len(waves) - len(extra))
                span_w = len(waves) - w0
                for j, pc in enumerate(extra):
                    sched[w0 + j * span_w // len(extra)].append(pc)

            ctx_tiles = {}
            ctx_queue = []
            for w, (pair, kb) in enumerate(waves):
                if kb == 0:
                    ctx_tiles[pair] = [
                        ps.tile([65, QC], FP32, tag="ctx", bufs=2,
                                name=f"c_{sc}_{pair}_{hh}")
                        for hh in range(2)]
                ex = wave_scores(sc, pair, kb)
                if pending_norms and w in (2, 3, nkb + 1, nkb + 2):
                    # deferred normalizations (prev chunk's pair 1 at waves
                    # 2-3, this chunk's pair 0 at waves nkb+1/nkb+2): emitted
                    # a few waves past their pair's end so their DVE/Pool
                    # chain pipelines behind subsequent waves instead of
                    # head-of-line-blocking proj combines (whose PSUM slot
                    # release gates the PE) right at the pair boundary.
                    pending_norms.pop(0)()
                for pc in sched[w]:
                    pc()
                # defer each pair's first two ctx matmuls by two waves: the
                # pair's ctx PSUM slots are recycled from the predecessor
                # pair, whose normalization staging copy needs a couple of
                # waves to execute - emitting ctx immediately would stall
                # the in-order PE on the slot.
                ctx_queue.append((pair, kb, ex))
                lag = 4 if kb < 4 else 0
                while len(ctx_queue) > lag or \
                        (ctx_queue and kb == nkb - 1):
                    qpair, qkb, qex = ctx_queue.pop(0)
                    wave_ctx(sc, qpair, qkb, qex, ctx_tiles[qpair], nkb)
                if kb == nkb - 1:
                    if pair == 1 and sc + 1 < NSC:
                        pending_norms = [
                            (lambda sc=sc, h=2 * pair + hh,
                                    t=ctx_tiles[pair][hh]:
                             emit_norm(sc, h, t))
                            for hh in range(2)]
                    elif pair == 0:
                        for hh in range(2):
                            emit_norm(sc, 2 * pair + hh, ctx_tiles[pair][hh])
                    elif sc != NSC - 1:
                        for hh in range(2):
                            emit_norm(sc, 2 * pair + hh, ctx_tiles[pair][hh])
                    else:
                        last_ctx = ctx_tiles[pair]
        # tail: the final pair's normalization is split by q-subblock and
        # interleaved with that q-block's out-projection, so each outproj
        # starts as soon as its own 128 columns of ctxT are normalized
        # instead of waiting for the full-width norm chain. Denominator
        # reciprocals run full-width once per head; the two heads' muls
        # alternate DVE/Pool so the chains pipeline; norms lead outproj by
        # one q-block.
        opcs = outproj_pieces(NSC - 1)
        for qq in range(5):
            if qq < 4:
                for hh in range(2):
                    emit_norm(NSC - 1, 2 + hh, last_ctx[hh], stage=False,
                              cols=(qq * 128, 128))
            if qq >= 1:
                opcs[2 * (qq - 1)]()
                opcs[2 * (qq - 1) + 1]()


def build_module():
    nc = bacc.Bacc("TRN2", target_bir_lowering=False, debug=False)
    xh = nc.dram_tensor("xh", [E, S], FP8, kind="ExternalInput").ap()
    xl = nc.dram_tensor("xl", [E, S], FP8, kind="ExternalInput").ap()
    wt = {nm: nc.dram_tensor(nm, [128, 8 * 256], FP8,
                             kind="ExternalInput").ap()
          for nm in ("whq", "whk", "whv", "wlq", "wlk", "wlv")}
    wo = nc.dram_tensor("wo", [M, E], BF16, kind="ExternalInput").ap()
    out = nc.dram_tensor("out", [S, E], BF16, kind="ExternalOutput").ap()
    with tile.TileContext(nc) as tc:
        _emit_kernel(tc, xh, xl, wt["whq"], wt["whk"], wt["whv"],
                     wt["wlq"], wt["wlk"], wt["wlv"], wo, out)
    nc.compile()
    return nc


def _split_fp8(a):
    """a -> (hi, lo) fp8 with a ~= hi + lo (lo at natural scale)."""
    f8 = ml_dtypes.float8_e4m3
    a = np.asarray(a, np.float32)
    hi = a.astype(f8)
    lo = (a - hi.astype(np.float32)).astype(f8)
    return np.ascontiguousarray(hi), np.ascontiguousarray(lo)


def make_in_maps(x, w_qkv):
    """Per-core input dicts (fp8 hi/lo splits, pre-transposed host-side)."""
    x = np.asarray(x, np.float32)
    w_qkv = np.asarray(w_qkv, np.float32)
    xsp = [_split_fp8(x[b].T) for b in range(B)]
    in_maps = []
    for c in range(NCORES):
        b, g = c // 4, c % 4
        cols = slice(g * M, (g + 1) * M)
        m = {"xh": xsp[b][0], "xl": xsp[b][1], "wo": None}
        for o, kind in enumerate("qkv"):
            # [E, 256] -> p-major [128, 8*256] so the DMA moves contiguous
            # 2KB-per-partition lines
            wt = (WS * w_qkv[o * E:][cols, :].T).astype(np.float32)
            pm = wt.reshape(8, 128, 256).transpose(1, 0, 2).reshape(128, -1)
            hi, lo = _split_fp8(pm)
            m["wh" + kind] = hi
            m["wl" + kind] = lo
        in_maps.append(m)
    return in_maps


_RUNNER = None
_SHARDED = None


def _get_runner():
    """Build the Bass module once and return a cached callable
    (in_maps) -> [NCORES, S, E] fp32 partial outputs."""
    global _RUNNER
    if _RUNNER is not None:
        return _RUNNER

    nc = build_module()

    from concourse import bass2jax
    import jax
    from jax.sharding import Mesh, PartitionSpec
    from jax.experimental.shard_map import shard_map

    bass2jax.install_neuronx_cc_hook()

    in_names = ["xh", "xl", "whq", "whk", "whv", "wlq", "wlk", "wlv", "wo"]
    out_names = ["out"]
    out_avals = [jax.core.ShapedArray((S, E), ml_dtypes.bfloat16)]
    n_params = len(in_names)
    all_names = in_names + out_names
    partition_name = (nc.partition_id_tensor.name
                      if nc.partition_id_tensor is not None else None)
    if partition_name is not None:
        all_names = all_names + [partition_name]

    def _body(*args):
        operands = list(args)
        if partition_name is not None:
            operands.append(bass2jax.partition_id_tensor())
        outs = bass2jax._bass_exec_p.bind(
            *operands,
            out_avals=tuple(out_avals),
            in_names=tuple(all_names),
            out_names=tuple(out_names),
            lowering_input_output_aliases=(),
            sim_require_finite=True,
            sim_require_nnan=True,
            nc=nc,
        )
        return tuple(outs)

    devices = jax.devices()[:NCORES]
    mesh = Mesh(np.asarray(devices), ("core",))
    n_outs = len(out_names)
    in_specs = (PartitionSpec("core"),) * (n_params + n_outs)
    out_specs = (PartitionSpec("core"),) * n_outs
    sharded = jax.jit(
        shard_map(_body, mesh=mesh, in_specs=in_specs, out_specs=out_specs,
                  check_rep=False),
        donate_argnums=tuple(range(n_params, n_params + n_outs)),
        keep_unused=True,
    )
    global _SHARDED
    _SHARDED = sharded

    def run(in_maps):
        concat_in = [
            np.concatenate([np.asarray(in_maps[c][n]) for c in range(NCORES)],
                           axis=0)
            for n in in_names
        ]
        concat_zeros = [np.zeros((NCORES * S, E), ml_dtypes.bfloat16)]
        out_arrs = sharded(*concat_in, *concat_zeros)
        return np.asarray(out_arrs[0]).astype(np.float32).reshape(NCORES, S, E)

    _RUNNER = run
    return run


def kernel(x, w_qkv, w_out, b_out):
    x = np.asarray(x, dtype=np.float32)
    w_qkv = np.asarray(w_qkv, dtype=np.float32)
    w_out = np.asarray(w_out, dtype=np.float32)
    b_out = np.asarray(b_out, dtype=np.float32)

    bf = ml_dtypes.bfloat16
    in_maps = make_in_maps(x, w_qkv)
    for c in range(NCORES):
        g = c % 4
        cols = slice(g * M, (g + 1) * M)
        # qTt/kTt/v1 carry the WS=32 fp8 pre-scale; scores fold 32*32 into
        # the exp scale, but ctxT is 32*ctx, so wo absorbs the 1/32 (exact
        # power-of-2 in bf16).
        in_maps[c]["wo"] = np.ascontiguousarray(
            w_out[:, cols].T / WS).astype(bf)

    run = _get_runner()
    partials = run(in_maps)  # [8, S, E] fp32

    out = np.empty((B, S, E), np.float32)
    for b in range(B):
        acc = partials[4 * b].astype(np.float64)
        for i in range(1, 4):
            acc += partials[4 * b + i]
        out[b] = (acc + b_out.astype(np.float64)).astype(np.float32)
    return out
